# revision 8
# baseline (speedup 1.0000x reference)
"""Trainium2 Bass kernel for nn_Adapter (ViT video adapter block).

Reference computation (per clip of T=16 frames, 14x14 spatial, 768 ch):
  h   = fc1(x_tokens)                                  # [3136, 384]
  g   = (dw3d_311(h) + dw3d_133(h) + dw3d_333(h))/3 + h
  f   = g + dw3d_311(g)            (proj)
  out = x_tokens + fc2(f)
CLS token passes through unchanged.

Strategy: data-parallel over the 8 clips (B=8), one clip per NeuronCore.
On-chip layout is channel-major [Ca, T*H*W] so the depthwise convs become
per-partition shift-and-MAC chains; the three 3D convs of stage 1 merge
into one 27-tap kernel (+identity), proj is 3 taps along T (+identity).
Spatial data sits in a zero-halo padded [18,16,16] flat layout; a shifted
tap is just a flat window offset (edge reads land in zero halos).  Taps are
split between VectorE (fused scalar_tensor_tensor MACs, bf16 2x mode — odd
offsets read a one-element-shifted copy P1b to stay 4B-aligned) and
TensorE (diag(w) matmuls accumulating in PSUM).  fc1/fc2 run on TensorE in
bf16; the residual x-add rides the fc2 PSUM group as an identity matmul.
"""

import os
import sys

sys.path.insert(0, "/opt/trn_rl_repo")

import numpy as np
import ml_dtypes

import concourse.bass as bass
import concourse.mybir as mybir
from concourse import bacc
from concourse.tile import TileContext
from concourse.bass_utils import run_bass_kernel_spmd
from concourse.masks import make_identity

BF16 = ml_dtypes.bfloat16

# Problem geometry (hardcoded; kernel must be self-contained).
B, T, L, C, CA = 8, 16, 197, 768, 384
H = W = 14
HW = H * W            # 196
TOK = T * HW          # 3136 tokens per clip
NCORES = 8
# padded conv layout: [Tp, Hp, Wp] flat with zero halos
TP, HP, WP = 18, 16, 16
NROW = TP * HP        # 288 rows of 16
PADSZ = TP * HP * WP  # 4608
PAD0 = 288            # lead/trail zero pad so shifted windows stay in-bounds
PADE = PAD0 + PADSZ + PAD0  # 5184 allocated
# fc1 output chunking: 392 tokens = 2 t-planes per PSUM bank
NCH, CHT = 8, 392

F32 = mybir.dt.float32
BF = mybir.dt.bfloat16

# stage-1 tap enumeration (27 taps)
TAPS1 = [(dt, dh, dw) for dt in (-1, 0, 1) for dh in (-1, 0, 1) for dw in (-1, 0, 1)]
TAP0 = TAPS1.index((0, 0, 0))

# ---- engine split config ----
# stage-1 tap indices computed on TensorE as diag(w) matmuls (must include
# TAP0 when non-empty so the bias rides the PSUM->SBUF merge).
N_PE_TAPS = int(os.environ.get("KN_PE_TAPS", "12"))
S_PE = ([TAP0] + [i for i in range(27) if i != TAP0])[:N_PE_TAPS]

_CACHE = {}

TRACE = os.environ.get("BASS_KERNEL_TRACE", "0") == "1"
LAST_EXEC_NS = None
LAST_RESULTS = None


def _win(ap, off, lo=0, hi=PADSZ):
    """Flat shifted window [128, hi-lo] of a [128, PADE] padded tile."""
    return ap[:, PAD0 + off + lo:PAD0 + off + hi]


def _rows(ap, off):
    """Shifted window as [128, NROW, 14] (skips the 2 halo cols per row)."""
    w = _win(ap, off)
    return w.rearrange("p (r w) -> p r w", r=NROW, w=WP)[:, :, 2:16]


def _build_graph():
    nc = bacc.Bacc("TRN2", target_bir_lowering=False, debug=False,
                   num_devices=NCORES)

    xT = nc.dram_tensor("xT", [C, TOK], BF, kind="ExternalInput").ap()
    xtok = nc.dram_tensor("xtok", [TOK, C], BF, kind="ExternalInput").ap()
    fc1_wT = nc.dram_tensor("fc1_wT", [C, CA], BF, kind="ExternalInput").ap()
    fc2_wT = nc.dram_tensor("fc2_wT", [CA, C], BF, kind="ExternalInput").ap()
    taps1 = nc.dram_tensor("taps1", [3, 128, 27], F32, kind="ExternalInput").ap()
    bias1 = nc.dram_tensor("bias1", [3, 128, 1], F32, kind="ExternalInput").ap()
    taps2 = nc.dram_tensor("taps2", [3, 128, 3], F32, kind="ExternalInput").ap()
    bias2 = nc.dram_tensor("bias2", [3, 128, 1], F32, kind="ExternalInput").ap()
    out = nc.dram_tensor("out", [TOK, C], F32, kind="ExternalOutput").ap()

    mult = mybir.AluOpType.mult
    add = mybir.AluOpType.add
    IDENT = mybir.ActivationFunctionType.Identity

    use_pe = len(S_PE) > 0
    s_dve = [i for i in range(27) if i not in S_PE]

    with TileContext(nc) as tc:
        with (
            tc.tile_pool(name="persist", bufs=1) as pp,
            tc.tile_pool(name="xm", bufs=3) as xmp,
            tc.tile_pool(name="outs", bufs=3) as outp,
            tc.tile_pool(name="ps1", bufs=2, space="PSUM") as ps1p,
            tc.tile_pool(name="ps2", bufs=3, space="PSUM") as ps2p,
            tc.tile_pool(name="psc", bufs=2, space="PSUM") as pscp,
        ):
            # ---- load constants ----
            fc1w_sb = []
            for k in range(6):
                t = pp.tile([128, CA], BF, tag=f"fc1w{k}", name=f"fc1w{k}")
                nc.sync.dma_start(out=t[:], in_=fc1_wT[k * 128:(k + 1) * 128, :])
                fc1w_sb.append(t)
            fc2w_sb = []
            for k in range(3):
                t = pp.tile([128, C], BF, tag=f"fc2w{k}", name=f"fc2w{k}")
                nc.sync.dma_start(out=t[:], in_=fc2_wT[k * 128:(k + 1) * 128, :])
                fc2w_sb.append(t)
            t1_sb, b1_sb, t2_sb, b2_sb = [], [], [], []
            for j in range(3):
                a = pp.tile([128, 27], F32, tag=f"t1_{j}", name=f"t1_{j}")
                nc.sync.dma_start(out=a[:], in_=taps1[j])
                t1_sb.append(a)
                a = pp.tile([128, 1], F32, tag=f"b1_{j}", name=f"b1_{j}")
                nc.sync.dma_start(out=a[:], in_=bias1[j])
                b1_sb.append(a)
                a = pp.tile([128, 3], F32, tag=f"t2_{j}", name=f"t2_{j}")
                nc.sync.dma_start(out=a[:], in_=taps2[j])
                t2_sb.append(a)
                a = pp.tile([128, 1], F32, tag=f"b2_{j}", name=f"b2_{j}")
                nc.sync.dma_start(out=a[:], in_=bias2[j])
                b2_sb.append(a)

            # identity (for the residual x-add matmul) + diag tap matrices
            ident = pp.tile([128, 128], BF, tag="ident", name="ident")
            make_identity(nc, ident[:])
            dg = {}
            for j in range(3):
                for ti in S_PE:
                    d = pp.tile([128, 128], BF, tag=f"dg{j}_{ti}",
                                name=f"dg{j}_{ti}")
                    nc.gpsimd.tensor_scalar_mul(
                        d[:], ident[:], t1_sb[j][:, ti:ti + 1])
                    dg[(j, ti)] = d

            # ---- load xT (channel-major tokens) ----
            xT_sb = []
            for k in range(6):
                t = pp.tile([128, TOK], BF, tag=f"xT{k}", name=f"xT{k}")
                nc.sync.dma_start(out=t[:], in_=xT[k * 128:(k + 1) * 128, :])
                xT_sb.append(t)

            # ---- conv buffers (zero halos) ----
            P1 = [pp.tile([128, PADE], BF, tag=f"P1_{j}", name=f"P1_{j}")
                  for j in range(3)]
            P1b = [pp.tile([128, PADE], BF, tag=f"P1b_{j}", name=f"P1b_{j}")
                   for j in range(3)]
            P2 = [pp.tile([128, PADE], BF, tag=f"P2_{j}", name=f"P2_{j}")
                  for j in range(3)]
            Fp = [pp.tile([128, PADSZ], BF, tag=f"Fp_{j}", name=f"Fp_{j}")
                  for j in range(3)]
            Ft = [pp.tile([128, TOK], BF, tag=f"Ft_{j}", name=f"Ft_{j}")
                  for j in range(3)]
            for j in range(3):
                nc.gpsimd.memset(P1[j][:], 0.0)
                nc.gpsimd.memset(P1b[j][:], 0.0)
                nc.gpsimd.memset(P2[j][:], 0.0)

            # ---- fc1: h[ca, tok] = fc1_wT.T @ xT, into P1 interior ----
            for j in range(3):
                for n in range(NCH):
                    ps = ps1p.tile([128, CHT], F32)
                    for k in range(6):
                        nc.tensor.matmul(
                            ps[:],
                            fc1w_sb[k][:, j * 128:(j + 1) * 128],
                            xT_sb[k][:, n * CHT:(n + 1) * CHT],
                            start=(k == 0), stop=(k == 5),
                        )
                    # evacuate into padded interior (2 t-planes), cast bf16
                    for q in range(2):
                        tpl = 2 * n + q
                        r0 = 16 * (tpl + 1) + 1
                        dst = _win(P1[j][:], 0).rearrange(
                            "p (r w) -> p r w", r=NROW, w=WP)[
                            :, r0:r0 + 14, 2:16]
                        src = ps[:, q * HW:(q + 1) * HW].rearrange(
                            "p (h w) -> p h w", h=H, w=W)
                        nc.scalar.copy(dst, src)
                # P1b = P1 shifted left one element: odd-offset taps read it
                # at even bases, keeping the DVE 2x perf mode.
                nc.gpsimd.tensor_copy(out=P1b[j][:, 0:PADE - 1],
                                      in_=P1[j][:, 1:PADE])

            # ---- stage 1 conv: P2 = conv27(P1) + bias1 (identity folded) --
            for j in range(3):
                if use_pe:
                    # PE partial: diag(w) matmuls accumulate S_PE taps in
                    # PSUM over 9 x 512 chunks; ACT merges (+bias1) into P2.
                    for c in range(9):
                        pc = pscp.tile([128, 512], F32)
                        for i, ti in enumerate(S_PE):
                            dt, dh, dw = TAPS1[ti]
                            off = dt * 256 + dh * 16 + dw
                            nc.tensor.matmul(
                                pc[:], dg[(j, ti)][:],
                                _win(P1[j][:], off, 512 * c, 512 * (c + 1)),
                                start=(i == 0), stop=(i == len(S_PE) - 1),
                            )
                        nc.scalar.activation(
                            _win(P2[j][:], 0, 512 * c, 512 * (c + 1)), pc[:],
                            IDENT, bias=b1_sb[j][:], scale=1.0)
                    first_dve = False
                else:
                    first_dve = True
                acc = _rows(P2[j][:], 0)
                for idx in s_dve:
                    dt, dh, dw = TAPS1[idx]
                    off = dt * 256 + dh * 16 + dw
                    if off % 2 == 0:
                        src = _rows(P1[j][:], off)
                    else:
                        src = _rows(P1b[j][:], off - 1)
                    if first_dve:
                        # acc := src*w + bias1 (broadcast) seeds the chain
                        nc.vector.scalar_tensor_tensor(
                            acc, src, t1_sb[j][:, idx:idx + 1],
                            b1_sb[j][:, 0:1].broadcast_to([128, NROW, 14]),
                            op0=mult, op1=add)
                        first_dve = False
                    else:
                        nc.vector.scalar_tensor_tensor(
                            acc, src, t1_sb[j][:, idx:idx + 1], acc,
                            op0=mult, op1=add)
                # stage 2 reads P2 at +-256: re-zero the t=0 / t=17 planes
                nc.gpsimd.memset(_win(P2[j][:], 0, 0, 256), 0.0)
                nc.gpsimd.memset(_win(P2[j][:], 0, 17 * 256, PADSZ), 0.0)

            # ---- stage 2: Fp = conv3_T(P2) + bias2 (identity folded) ----
            for j in range(3):
                Fw = Fp[j][:].rearrange("p (r w) -> p r w", r=NROW, w=WP)[
                    :, :, 2:16]
                nc.vector.scalar_tensor_tensor(
                    Fw, _rows(P2[j][:], 0), t2_sb[j][:, 1:2],
                    b2_sb[j][:, 0:1].broadcast_to([128, NROW, 14]),
                    op0=mult, op1=add)
                for dt in (-1, 1):
                    src = _rows(P2[j][:], dt * 256)
                    nc.vector.scalar_tensor_tensor(
                        Fw, src, t2_sb[j][:, dt + 1:dt + 2], Fw,
                        op0=mult, op1=add)
                # compact padded -> tight tokens (gpsimd, off critical path)
                for tpl in range(T):
                    r0 = 16 * (tpl + 1) + 1
                    src = Fp[j][:].rearrange(
                        "p (r w) -> p r w", r=NROW, w=WP)[
                        :, r0:r0 + 14, 2:16]
                    dst = Ft[j][:, tpl * HW:(tpl + 1) * HW].rearrange(
                        "p (h w) -> p h w", h=H, w=W)
                    nc.gpsimd.tensor_copy(out=dst, in_=src)

            # ---- fc2 + residual add (identity matmul) + store ----
            m_tiles = [(m * 128, 128) for m in range(24)] + [(3072, 64)]
            for (m0, M) in m_tiles:
                xm = xmp.tile([128, C], BF)
                nc.sync.dma_start(out=xm[:M], in_=xtok[m0:m0 + M, :])
                ot = outp.tile([128, C], F32)
                for nh in range(2):
                    ps = ps2p.tile([128, 384], F32)
                    for k in range(3):
                        nc.tensor.matmul(
                            ps[:M],
                            Ft[k][:, m0:m0 + M],
                            fc2w_sb[k][:, nh * 384:(nh + 1) * 384],
                            start=(k == 0), stop=False,
                        )
                    nc.tensor.matmul(
                        ps[:M], ident[:M, :M],
                        xm[:M, nh * 384:(nh + 1) * 384],
                        start=False, stop=True,
                    )
                    nc.scalar.copy(ot[:M, nh * 384:(nh + 1) * 384], ps[:M])
                nc.sync.dma_start(out=out[m0:m0 + M, :], in_=ot[:M])

    nc.compile()
    return nc


def _prep_shared(fc1_w, fc1_b, conv1_w, conv1_b, conv2_w, conv2_b,
                 conv3_w, conv3_b, proj_w, proj_b, fc2_w, fc2_b):
    assert not np.any(fc1_b), "nonzero fc1_b not supported by this build"
    # merged stage-1 kernel: (c1 + c2 + c3)/3 + identity
    w_eff = np.array(conv3_w[:, 0], dtype=np.float64)            # [CA,3,3,3]
    w_eff[:, :, 1, 1] += conv1_w[:, 0, :, 0, 0]
    w_eff[:, 1, :, :] += conv2_w[:, 0, 0, :, :]
    w_eff /= 3.0
    w_eff[:, 1, 1, 1] += 1.0
    b_eff = (conv1_b + conv2_b + conv3_b) / 3.0
    # stage-2 (proj) taps along T + identity
    a_eff = np.array(proj_w[:, 0, :, 0, 0], dtype=np.float64)    # [CA,3]
    a_eff[:, 1] += 1.0

    taps1 = np.empty((3, 128, 27), np.float32)
    for idx, (dt, dh, dw) in enumerate(TAPS1):
        taps1[:, :, idx] = w_eff[:, dt + 1, dh + 1, dw + 1].reshape(3, 128)
    bias1 = np.asarray(b_eff, np.float32).reshape(3, 128, 1)
    taps2 = np.asarray(a_eff, np.float32).reshape(3, 128, 3)
    bias2 = np.asarray(proj_b, np.float32).reshape(3, 128, 1)

    fc1_wT = np.ascontiguousarray(np.asarray(fc1_w, np.float32).T).astype(BF16)
    fc2_wT = np.ascontiguousarray(np.asarray(fc2_w, np.float32).T).astype(BF16)
    return dict(fc1_wT=fc1_wT, fc2_wT=fc2_wT, taps1=taps1, bias1=bias1,
                taps2=taps2, bias2=bias2), np.asarray(fc2_b, np.float32)


def kernel(x, fc1_w, fc1_b, conv1_w, conv1_b, conv2_w, conv2_b,
           conv3_w, conv3_b, proj_w, proj_b, fc2_w, fc2_b, T=16):
    global LAST_EXEC_NS, LAST_RESULTS
    x = np.asarray(x, np.float32)
    Tv = int(np.asarray(T))
    assert Tv == 16 and x.shape == (B * Tv, L, C), (Tv, x.shape)

    if "nc" not in _CACHE:
        _CACHE["nc"] = _build_graph()
    nc = _CACHE["nc"]

    shared, fc2_b_np = _prep_shared(
        np.asarray(fc1_w, np.float32), np.asarray(fc1_b, np.float32),
        np.asarray(conv1_w, np.float32), np.asarray(conv1_b, np.float32),
        np.asarray(conv2_w, np.float32), np.asarray(conv2_b, np.float32),
        np.asarray(conv3_w, np.float32), np.asarray(conv3_b, np.float32),
        np.asarray(proj_w, np.float32), np.asarray(proj_b, np.float32),
        np.asarray(fc2_w, np.float32), np.asarray(fc2_b, np.float32))

    in_maps = []
    for i in range(NCORES):
        clip = x[i * Tv:(i + 1) * Tv]                    # [16, 197, 768]
        tokens = np.ascontiguousarray(clip[:, 1:, :]).reshape(TOK, C)
        m = dict(shared)
        m["xT"] = np.ascontiguousarray(tokens.T).astype(BF16)
        m["xtok"] = (tokens + fc2_b_np[None, :]).astype(BF16)
        in_maps.append(m)

    res = run_bass_kernel_spmd(nc, in_maps, core_ids=list(range(NCORES)),
                               trace=TRACE)
    LAST_EXEC_NS = res.exec_time_ns
    LAST_RESULTS = res

    full = np.array(x)  # CLS rows (and everything) start as x
    for i in range(NCORES):
        h = res.results[i]["out"].reshape(Tv, HW, C)
        full[i * Tv:(i + 1) * Tv, 1:, :] = h
    return full


# revision 9
# speedup vs baseline: 1.4920x; 1.4920x over previous
"""Trainium2 Bass kernel for nn_Adapter (ViT video adapter block).

Reference computation (per clip of T=16 frames, 14x14 spatial, 768 ch):
  h   = fc1(x_tokens)                                  # [3136, 384]
  g   = (dw3d_311(h) + dw3d_133(h) + dw3d_333(h))/3 + h
  f   = g + dw3d_311(g)            (proj)
  out = x_tokens + fc2(f)
CLS token passes through unchanged.

Strategy: data-parallel over the 8 clips (B=8), one clip per NeuronCore.
On-chip layout is channel-major [Ca, T*H*W] so the depthwise convs become
per-partition shift-and-MAC chains; the three 3D convs of stage 1 merge
into one 27-tap kernel (+identity), proj is 3 taps along T (+identity).
Spatial data sits in a zero-halo padded [18,16,16] flat layout; a shifted
tap is just a flat window offset (edge reads land in zero halos).  Taps are
split between VectorE (fused scalar_tensor_tensor MACs, bf16 2x mode — odd
offsets read a one-element-shifted copy P1b to stay 4B-aligned) and
TensorE (diag(w) matmuls accumulating in PSUM).  fc1/fc2 run on TensorE in
bf16; the residual x-add rides the fc2 PSUM group as an identity matmul.
"""

import os
import sys

sys.path.insert(0, "/opt/trn_rl_repo")

import numpy as np
import ml_dtypes

import concourse.bass as bass
import concourse.mybir as mybir
from concourse import bacc
from concourse.tile import TileContext
from concourse.bass_utils import run_bass_kernel_spmd
from concourse.masks import make_identity

BF16 = ml_dtypes.bfloat16

# Problem geometry (hardcoded; kernel must be self-contained).
B, T, L, C, CA = 8, 16, 197, 768, 384
H = W = 14
HW = H * W            # 196
TOK = T * HW          # 3136 tokens per clip
NCORES = 8
# padded conv layout: [Tp, Hp, Wp] flat with zero halos
TP, HP, WP = 18, 16, 16
NROW = TP * HP        # 288 rows of 16
PADSZ = TP * HP * WP  # 4608
PAD0 = 288            # lead/trail zero pad so shifted windows stay in-bounds
PADE = PAD0 + PADSZ + PAD0  # 5184 allocated
# fc1 output chunking: 392 tokens = 2 t-planes per PSUM bank
NCH, CHT = 8, 392

F32 = mybir.dt.float32
BF = mybir.dt.bfloat16

# stage-1 tap enumeration (27 taps)
TAPS1 = [(dt, dh, dw) for dt in (-1, 0, 1) for dh in (-1, 0, 1) for dw in (-1, 0, 1)]
TAP0 = TAPS1.index((0, 0, 0))

# ---- engine split config ----
# stage-1 tap indices computed on TensorE as diag(w) matmuls (must include
# TAP0 when non-empty so the bias rides the PSUM->SBUF merge).
N_PE_TAPS = int(os.environ.get("KN_PE_TAPS", "14"))
S_PE = ([TAP0] + [i for i in range(27) if i != TAP0])[:N_PE_TAPS]

_CACHE = {}

TRACE = os.environ.get("BASS_KERNEL_TRACE", "0") == "1"
LAST_EXEC_NS = None
LAST_RESULTS = None


def _win(ap, off, lo=0, hi=PADSZ):
    """Flat shifted window [128, hi-lo] of a [128, PADE] padded tile."""
    return ap[:, PAD0 + off + lo:PAD0 + off + hi]


def _rows(ap, off):
    """Shifted window as [128, NROW, 14] (skips the 2 halo cols per row)."""
    w = _win(ap, off)
    return w.rearrange("p (r w) -> p r w", r=NROW, w=WP)[:, :, 2:16]


def _build_graph():
    nc = bacc.Bacc("TRN2", target_bir_lowering=False, debug=False,
                   num_devices=NCORES)

    xT = nc.dram_tensor("xT", [C, TOK], BF, kind="ExternalInput").ap()
    xtok = nc.dram_tensor("xtok", [TOK, C], BF, kind="ExternalInput").ap()
    fc1_wT = nc.dram_tensor("fc1_wT", [C, CA], BF, kind="ExternalInput").ap()
    fc2_wT = nc.dram_tensor("fc2_wT", [CA, C], BF, kind="ExternalInput").ap()
    taps1 = nc.dram_tensor("taps1", [3, 128, 27], F32, kind="ExternalInput").ap()
    bias1 = nc.dram_tensor("bias1", [3, 128, 1], F32, kind="ExternalInput").ap()
    bias2 = nc.dram_tensor("bias2", [3, 128, 1], F32, kind="ExternalInput").ap()
    diag1 = nc.dram_tensor("diag1", [3, 27, 128, 128], BF,
                           kind="ExternalInput").ap()
    diag2 = nc.dram_tensor("diag2", [3, 3, 128, 128], BF,
                           kind="ExternalInput").ap()
    out = nc.dram_tensor("out", [TOK, C], F32, kind="ExternalOutput").ap()

    mult = mybir.AluOpType.mult
    add = mybir.AluOpType.add
    IDENT = mybir.ActivationFunctionType.Identity

    use_pe = len(S_PE) > 0
    s_dve = [i for i in range(27) if i not in S_PE]
    # conv "region": planes 1..16 of the padded layout = [256, 4352) flat,
    # 8 chunks of 512 (all interior tokens + in-plane halo rows/cols).
    REG0 = 256
    NREGROW = 256  # rows 16..272

    def regwin(ap, off, c):
        lo = PAD0 + off + REG0 + 512 * c
        return ap[:, lo:lo + 512]

    def regrows(ap, off):
        w = ap[:, PAD0 + off + REG0:PAD0 + off + REG0 + 4096]
        return w.rearrange("p (r w) -> p r w", r=NREGROW, w=WP)[:, :, 2:16]

    with TileContext(nc) as tc:
        with (
            tc.tile_pool(name="persist", bufs=1) as pp,
            tc.tile_pool(name="xm", bufs=3) as xmp,
            tc.tile_pool(name="outs", bufs=3) as outp,
            tc.tile_pool(name="ps1", bufs=2, space="PSUM") as ps1p,
            tc.tile_pool(name="ps2", bufs=3, space="PSUM") as ps2p,
            tc.tile_pool(name="psc", bufs=3, space="PSUM") as pscp,
        ):
            # ---- load constants ----
            fc1w_sb = []
            for k in range(6):
                t = pp.tile([128, CA], BF, tag=f"fc1w{k}", name=f"fc1w{k}")
                nc.sync.dma_start(out=t[:], in_=fc1_wT[k * 128:(k + 1) * 128, :])
                fc1w_sb.append(t)
            fc2w_sb = []
            for k in range(3):
                t = pp.tile([128, C], BF, tag=f"fc2w{k}", name=f"fc2w{k}")
                nc.sync.dma_start(out=t[:], in_=fc2_wT[k * 128:(k + 1) * 128, :])
                fc2w_sb.append(t)
            t1_sb, b1_sb, b2_sb = [], [], []
            for j in range(3):
                a = pp.tile([128, 27], F32, tag=f"t1_{j}", name=f"t1_{j}")
                nc.sync.dma_start(out=a[:], in_=taps1[j])
                t1_sb.append(a)
                a = pp.tile([128, 1], F32, tag=f"b1_{j}", name=f"b1_{j}")
                nc.sync.dma_start(out=a[:], in_=bias1[j])
                b1_sb.append(a)
                a = pp.tile([128, 1], F32, tag=f"b2_{j}", name=f"b2_{j}")
                nc.sync.dma_start(out=a[:], in_=bias2[j])
                b2_sb.append(a)

            # identity (residual x-add matmul) + host-built diag tap matrices
            ident = pp.tile([128, 128], BF, tag="ident", name="ident")
            make_identity(nc, ident[:])
            dg1, dg2 = {}, {}
            for j in range(3):
                for ti in S_PE:
                    d = pp.tile([128, 128], BF, tag=f"dg{j}_{ti}",
                                name=f"dg{j}_{ti}")
                    nc.sync.dma_start(out=d[:], in_=diag1[j, ti])
                    dg1[(j, ti)] = d
                for ti in range(3):
                    d = pp.tile([128, 128], BF, tag=f"dh{j}_{ti}",
                                name=f"dh{j}_{ti}")
                    nc.sync.dma_start(out=d[:], in_=diag2[j, ti])
                    dg2[(j, ti)] = d

            # ---- load xT (channel-major tokens) ----
            xT_sb = []
            for k in range(6):
                t = pp.tile([128, TOK], BF, tag=f"xT{k}", name=f"xT{k}")
                nc.sync.dma_start(out=t[:], in_=xT[k * 128:(k + 1) * 128, :])
                xT_sb.append(t)

            # ---- conv buffers (zero halos) ----
            P1 = [pp.tile([128, PADE], BF, tag=f"P1_{j}", name=f"P1_{j}")
                  for j in range(3)]
            P2 = [pp.tile([128, PADE], BF, tag=f"P2_{j}", name=f"P2_{j}")
                  for j in range(3)]
            Ft = [pp.tile([128, TOK], BF, tag=f"Ft_{j}", name=f"Ft_{j}")
                  for j in range(3)]
            for j in range(3):
                nc.vector.memset(P1[j][:], 0.0)
                nc.vector.memset(P2[j][:], 0.0)

            # ---- fc1: h[ca, tok] = fc1_wT.T @ xT, into P1 interior ----
            for j in range(3):
                for n in range(NCH):
                    ps = ps1p.tile([128, CHT], F32)
                    for k in range(6):
                        nc.tensor.matmul(
                            ps[:],
                            fc1w_sb[k][:, j * 128:(j + 1) * 128],
                            xT_sb[k][:, n * CHT:(n + 1) * CHT],
                            start=(k == 0), stop=(k == 5),
                        )
                    # evacuate into padded interior (2 t-planes), cast bf16
                    for q in range(2):
                        tpl = 2 * n + q
                        r0 = 16 * (tpl + 1) + 1
                        dst = _win(P1[j][:], 0).rearrange(
                            "p (r w) -> p r w", r=NROW, w=WP)[
                            :, r0:r0 + 14, 2:16]
                        src = ps[:, q * HW:(q + 1) * HW].rearrange(
                            "p (h w) -> p h w", h=H, w=W)
                        nc.scalar.copy(dst, src)

            # ---- stage 1 conv: P2 = conv27(P1) + bias1 (identity folded) --
            for j in range(3):
                if use_pe:
                    # PE partial: diag(w) matmuls accumulate S_PE taps in
                    # PSUM over 8 x 512 region chunks; ACT merges (+bias1).
                    for c in range(8):
                        pc = pscp.tile([128, 512], F32)
                        for i, ti in enumerate(S_PE):
                            dt, dh, dw = TAPS1[ti]
                            off = dt * 256 + dh * 16 + dw
                            nc.tensor.matmul(
                                pc[:], dg1[(j, ti)][:],
                                regwin(P1[j][:], off, c),
                                start=(i == 0), stop=(i == len(S_PE) - 1),
                            )
                        nc.scalar.activation(
                            regwin(P2[j][:], 0, c), pc[:],
                            IDENT, bias=b1_sb[j][:], scale=1.0)
                    first_dve = False
                else:
                    first_dve = True
                acc = regrows(P2[j][:], 0)
                for idx in s_dve:
                    dt, dh, dw = TAPS1[idx]
                    off = dt * 256 + dh * 16 + dw
                    src = regrows(P1[j][:], off)
                    if first_dve:
                        nc.vector.scalar_tensor_tensor(
                            acc, src, t1_sb[j][:, idx:idx + 1],
                            b1_sb[j][:, 0:1].broadcast_to([128, NREGROW, 14]),
                            op0=mult, op1=add)
                        first_dve = False
                    else:
                        nc.vector.scalar_tensor_tensor(
                            acc, src, t1_sb[j][:, idx:idx + 1], acc,
                            op0=mult, op1=add)

            # ---- stage 2 on PE: Ft = conv3_T(P2) + bias2, straight to
            # tight layout via per-plane PSUM merges ----
            for j in range(3):
                for c in range(8):
                    pc = pscp.tile([128, 512], F32)
                    for i, dt in enumerate((-1, 0, 1)):
                        nc.tensor.matmul(
                            pc[:], dg2[(j, dt + 1)][:],
                            regwin(P2[j][:], dt * 256, c),
                            start=(i == 0), stop=(i == 2),
                        )
                    for q in range(2):
                        tpl = 2 * c + q
                        dst = Ft[j][:, tpl * HW:(tpl + 1) * HW].rearrange(
                            "p (h w) -> p h w", h=H, w=W)
                        src = pc[:, q * 256:(q + 1) * 256].rearrange(
                            "p (h w) -> p h w", h=16, w=16)[:, 1:15, 2:16]
                        nc.scalar.activation(
                            dst, src, IDENT, bias=b2_sb[j][:], scale=1.0)

            # ---- fc2 + residual add (identity matmul) + store ----
            m_tiles = [(m * 128, 128) for m in range(24)] + [(3072, 64)]
            for (m0, M) in m_tiles:
                xm = xmp.tile([128, C], BF)
                nc.sync.dma_start(out=xm[:M], in_=xtok[m0:m0 + M, :])
                ot = outp.tile([128, C], F32)
                for nh in range(2):
                    ps = ps2p.tile([128, 384], F32)
                    for k in range(3):
                        nc.tensor.matmul(
                            ps[:M],
                            Ft[k][:, m0:m0 + M],
                            fc2w_sb[k][:, nh * 384:(nh + 1) * 384],
                            start=(k == 0), stop=False,
                        )
                    nc.tensor.matmul(
                        ps[:M], ident[:M, :M],
                        xm[:M, nh * 384:(nh + 1) * 384],
                        start=False, stop=True,
                    )
                    nc.scalar.copy(ot[:M, nh * 384:(nh + 1) * 384], ps[:M])
                nc.sync.dma_start(out=out[m0:m0 + M, :], in_=ot[:M])

    nc.compile()
    return nc


def _prep_shared(fc1_w, fc1_b, conv1_w, conv1_b, conv2_w, conv2_b,
                 conv3_w, conv3_b, proj_w, proj_b, fc2_w, fc2_b):
    assert not np.any(fc1_b), "nonzero fc1_b not supported by this build"
    # merged stage-1 kernel: (c1 + c2 + c3)/3 + identity
    w_eff = np.array(conv3_w[:, 0], dtype=np.float64)            # [CA,3,3,3]
    w_eff[:, :, 1, 1] += conv1_w[:, 0, :, 0, 0]
    w_eff[:, 1, :, :] += conv2_w[:, 0, 0, :, :]
    w_eff /= 3.0
    w_eff[:, 1, 1, 1] += 1.0
    b_eff = (conv1_b + conv2_b + conv3_b) / 3.0
    # stage-2 (proj) taps along T + identity
    a_eff = np.array(proj_w[:, 0, :, 0, 0], dtype=np.float64)    # [CA,3]
    a_eff[:, 1] += 1.0

    taps1 = np.empty((3, 128, 27), np.float32)
    for idx, (dt, dh, dw) in enumerate(TAPS1):
        taps1[:, :, idx] = w_eff[:, dt + 1, dh + 1, dw + 1].reshape(3, 128)
    bias1 = np.asarray(b_eff, np.float32).reshape(3, 128, 1)
    taps2 = np.asarray(a_eff, np.float32).reshape(3, 128, 3)
    bias2 = np.asarray(proj_b, np.float32).reshape(3, 128, 1)

    idx128 = np.arange(128)
    diag1 = np.zeros((3, 27, 128, 128), np.float32)
    diag1[:, :, idx128, idx128] = taps1.transpose(0, 2, 1)
    diag2 = np.zeros((3, 3, 128, 128), np.float32)
    diag2[:, :, idx128, idx128] = taps2.transpose(0, 2, 1)

    fc1_wT = np.ascontiguousarray(np.asarray(fc1_w, np.float32).T).astype(BF16)
    fc2_wT = np.ascontiguousarray(np.asarray(fc2_w, np.float32).T).astype(BF16)
    return dict(fc1_wT=fc1_wT, fc2_wT=fc2_wT, taps1=taps1, bias1=bias1,
                bias2=bias2, diag1=diag1.astype(BF16),
                diag2=diag2.astype(BF16)), np.asarray(fc2_b, np.float32)


def kernel(x, fc1_w, fc1_b, conv1_w, conv1_b, conv2_w, conv2_b,
           conv3_w, conv3_b, proj_w, proj_b, fc2_w, fc2_b, T=16):
    global LAST_EXEC_NS, LAST_RESULTS
    x = np.asarray(x, np.float32)
    Tv = int(np.asarray(T))
    assert Tv == 16 and x.shape == (B * Tv, L, C), (Tv, x.shape)

    if "nc" not in _CACHE:
        _CACHE["nc"] = _build_graph()
    nc = _CACHE["nc"]

    shared, fc2_b_np = _prep_shared(
        np.asarray(fc1_w, np.float32), np.asarray(fc1_b, np.float32),
        np.asarray(conv1_w, np.float32), np.asarray(conv1_b, np.float32),
        np.asarray(conv2_w, np.float32), np.asarray(conv2_b, np.float32),
        np.asarray(conv3_w, np.float32), np.asarray(conv3_b, np.float32),
        np.asarray(proj_w, np.float32), np.asarray(proj_b, np.float32),
        np.asarray(fc2_w, np.float32), np.asarray(fc2_b, np.float32))

    in_maps = []
    for i in range(NCORES):
        clip = x[i * Tv:(i + 1) * Tv]                    # [16, 197, 768]
        tokens = np.ascontiguousarray(clip[:, 1:, :]).reshape(TOK, C)
        m = dict(shared)
        m["xT"] = np.ascontiguousarray(tokens.T).astype(BF16)
        m["xtok"] = (tokens + fc2_b_np[None, :]).astype(BF16)
        in_maps.append(m)

    res = run_bass_kernel_spmd(nc, in_maps, core_ids=list(range(NCORES)),
                               trace=TRACE)
    LAST_EXEC_NS = res.exec_time_ns
    LAST_RESULTS = res

    full = np.array(x)  # CLS rows (and everything) start as x
    for i in range(NCORES):
        h = res.results[i]["out"].reshape(Tv, HW, C)
        full[i * Tv:(i + 1) * Tv, 1:, :] = h
    return full


# revision 10
# speedup vs baseline: 1.8169x; 1.2178x over previous
"""Trainium2 Bass kernel for nn_Adapter (ViT video adapter block).

Reference computation (per clip of T=16 frames, 14x14 spatial, 768 ch):
  h   = fc1(x_tokens)                                  # [3136, 384]
  g   = (dw3d_311(h) + dw3d_133(h) + dw3d_333(h))/3 + h
  f   = g + dw3d_311(g)            (proj)
  out = x_tokens + fc2(f)
CLS token passes through unchanged.

Strategy: data-parallel over the 8 clips (B=8), one clip per NeuronCore.
On-chip layout is channel-major [Ca, T*H*W] so the depthwise convs become
per-partition shift-and-MAC chains; the three 3D convs of stage 1 merge
into one 27-tap kernel (+identity), proj is 3 taps along T (+identity).
Spatial data sits in a zero-halo padded [18,16,16] flat layout; a shifted
tap is just a flat window offset (edge reads land in zero halos).  Taps are
split between VectorE (fused scalar_tensor_tensor MACs, bf16 2x mode — odd
offsets read a one-element-shifted copy P1b to stay 4B-aligned) and
TensorE (diag(w) matmuls accumulating in PSUM).  fc1/fc2 run on TensorE in
bf16; the residual x-add rides the fc2 PSUM group as an identity matmul.
"""

import os
import sys

sys.path.insert(0, "/opt/trn_rl_repo")

import numpy as np
import ml_dtypes

import concourse.bass as bass
import concourse.mybir as mybir
from concourse import bacc
from concourse.tile import TileContext
from concourse.bass_utils import run_bass_kernel_spmd
from concourse.masks import make_identity

BF16 = ml_dtypes.bfloat16

# Problem geometry (hardcoded; kernel must be self-contained).
B, T, L, C, CA = 8, 16, 197, 768, 384
H = W = 14
HW = H * W            # 196
TOK = T * HW          # 3136 tokens per clip
NCORES = 8
# padded conv layout: [Tp, Hp, Wp] flat with zero halos
TP, HP, WP = 18, 16, 16
NROW = TP * HP        # 288 rows of 16
PADSZ = TP * HP * WP  # 4608
PAD0 = 288            # lead/trail zero pad so shifted windows stay in-bounds
PADE = PAD0 + PADSZ + PAD0  # 5184 allocated
# fc1 output chunking: 392 tokens = 2 t-planes per PSUM bank
NCH, CHT = 8, 392

F32 = mybir.dt.float32
BF = mybir.dt.bfloat16

# stage-1 tap enumeration (27 taps)
TAPS1 = [(dt, dh, dw) for dt in (-1, 0, 1) for dh in (-1, 0, 1) for dw in (-1, 0, 1)]
TAP0 = TAPS1.index((0, 0, 0))

# ---- engine split config ----
# stage-1 tap indices computed on TensorE as diag(w) matmuls (must include
# TAP0 when non-empty so the bias rides the PSUM->SBUF merge).
N_PE_TAPS = int(os.environ.get("KN_PE_TAPS", "16"))
S_PE = ([TAP0] + [i for i in range(27) if i != TAP0])[:N_PE_TAPS]

_CACHE = {}

TRACE = os.environ.get("BASS_KERNEL_TRACE", "0") == "1"
LAST_EXEC_NS = None
LAST_RESULTS = None


def _win(ap, off, lo=0, hi=PADSZ):
    """Flat shifted window [128, hi-lo] of a [128, PADE] padded tile."""
    return ap[:, PAD0 + off + lo:PAD0 + off + hi]


def _rows(ap, off):
    """Shifted window as [128, NROW, 14] (skips the 2 halo cols per row)."""
    w = _win(ap, off)
    return w.rearrange("p (r w) -> p r w", r=NROW, w=WP)[:, :, 2:16]


def _build_graph():
    nc = bacc.Bacc("TRN2", target_bir_lowering=False, debug=False,
                   num_devices=NCORES)

    xT = nc.dram_tensor("xT", [C, TOK], BF, kind="ExternalInput").ap()
    xtok = nc.dram_tensor("xtok", [TOK, C], BF, kind="ExternalInput").ap()
    fc1_wT = nc.dram_tensor("fc1_wT", [C, CA], BF, kind="ExternalInput").ap()
    fc2_wT = nc.dram_tensor("fc2_wT", [CA, C], BF, kind="ExternalInput").ap()
    taps1 = nc.dram_tensor("taps1", [3, 128, 27], F32, kind="ExternalInput").ap()
    bias1 = nc.dram_tensor("bias1", [3, 128, 1], F32, kind="ExternalInput").ap()
    bias2 = nc.dram_tensor("bias2", [3, 128, 1], F32, kind="ExternalInput").ap()
    diag1 = nc.dram_tensor("diag1", [3, max(1, len(S_PE)), 128, 128], BF,
                           kind="ExternalInput").ap()
    diag2 = nc.dram_tensor("diag2", [3, 3, 128, 128], BF,
                           kind="ExternalInput").ap()
    out = nc.dram_tensor("out", [TOK, C], F32, kind="ExternalOutput").ap()

    mult = mybir.AluOpType.mult
    add = mybir.AluOpType.add
    IDENT = mybir.ActivationFunctionType.Identity

    use_pe = len(S_PE) > 0
    s_dve = [i for i in range(27) if i not in S_PE]
    # conv "region": planes 1..16 of the padded layout = [256, 4352) flat,
    # 8 chunks of 512 (all interior tokens + in-plane halo rows/cols).
    REG0 = 256

    def regwin(ap, off, c):
        lo = PAD0 + off + REG0 + 512 * c
        return ap[:, lo:lo + 512]

    def reghalf(ap, off, h):
        """Half-region rows view [128, 128, 14] (rows 128h..128h+128)."""
        lo = PAD0 + off + REG0 + 2048 * h
        w = ap[:, lo:lo + 2048]
        return w.rearrange("p (r w) -> p r w", r=128, w=WP)[:, :, 2:16]

    with TileContext(nc) as tc:
        with (
            tc.tile_pool(name="persist", bufs=1) as pp,
            tc.tile_pool(name="xm", bufs=3) as xmp,
            tc.tile_pool(name="outs", bufs=3) as outp,
            tc.tile_pool(name="ps1", bufs=2, space="PSUM") as ps1p,
            tc.tile_pool(name="ps2", bufs=3, space="PSUM") as ps2p,
            tc.tile_pool(name="psc", bufs=3, space="PSUM") as pscp,
        ):
            # ---- xT (channel-major tokens) + fc1 weights first: they gate
            # the first matmul ----
            xT_sb = []
            for k in range(6):
                t = pp.tile([128, TOK], BF, tag=f"xT{k}", name=f"xT{k}")
                nc.sync.dma_start(out=t[:, :TOK // 2],
                                  in_=xT[k * 128:(k + 1) * 128, :TOK // 2])
                nc.sync.dma_start(out=t[:, TOK // 2:],
                                  in_=xT[k * 128:(k + 1) * 128, TOK // 2:])
                xT_sb.append(t)
            fc1w_sb = []
            for k in range(6):
                t = pp.tile([128, CA], BF, tag=f"fc1w{k}", name=f"fc1w{k}")
                nc.sync.dma_start(out=t[:], in_=fc1_wT[k * 128:(k + 1) * 128, :])
                fc1w_sb.append(t)
            t1_sb, b1_sb, b2_sb = [], [], []
            for j in range(3):
                a = pp.tile([128, 27], F32, tag=f"t1_{j}", name=f"t1_{j}")
                nc.sync.dma_start(out=a[:], in_=taps1[j])
                t1_sb.append(a)
                a = pp.tile([128, 1], F32, tag=f"b1_{j}", name=f"b1_{j}")
                nc.sync.dma_start(out=a[:], in_=bias1[j])
                b1_sb.append(a)
                a = pp.tile([128, 1], F32, tag=f"b2_{j}", name=f"b2_{j}")
                nc.sync.dma_start(out=a[:], in_=bias2[j])
                b2_sb.append(a)

            # diag tap matrices (host-built), identity for the x-add
            dg1, dg2 = {}, {}
            for j in range(3):
                for i, ti in enumerate(S_PE):
                    d = pp.tile([128, 128], BF, tag=f"dg{j}_{ti}",
                                name=f"dg{j}_{ti}")
                    nc.sync.dma_start(out=d[:], in_=diag1[j, i])
                    dg1[(j, ti)] = d
                for ti in range(3):
                    d = pp.tile([128, 128], BF, tag=f"dh{j}_{ti}",
                                name=f"dh{j}_{ti}")
                    nc.sync.dma_start(out=d[:], in_=diag2[j, ti])
                    dg2[(j, ti)] = d
            ident = pp.tile([128, 128], BF, tag="ident", name="ident")
            make_identity(nc, ident[:])
            fc2w_sb = []
            for k in range(3):
                t = pp.tile([128, C], BF, tag=f"fc2w{k}", name=f"fc2w{k}")
                nc.sync.dma_start(out=t[:], in_=fc2_wT[k * 128:(k + 1) * 128, :])
                fc2w_sb.append(t)

            # ---- conv buffers; only halos need zeroing (ScalarE memzero,
            # DVE stays free) ----
            P1 = [pp.tile([128, PADE], BF, tag=f"P1_{j}", name=f"P1_{j}")
                  for j in range(3)]
            P2 = [pp.tile([128, PADE], BF, tag=f"P2_{j}", name=f"P2_{j}")
                  for j in range(3)]
            Ft = [pp.tile([128, TOK], BF, tag=f"Ft_{j}", name=f"Ft_{j}")
                  for j in range(3)]
            for j in range(3):
                nc.scalar.memzero(P1[j][:])
                # stage-2 windows only read P2 planes 0 / 17 outside the
                # written region
                nc.scalar.memzero(P2[j][:, PAD0:PAD0 + 256])
                nc.scalar.memzero(P2[j][:, PAD0 + 4352:PAD0 + 4608])

            # ---- per-ca-tile pipeline: fc1 -> conv stage 1 -> stage 2 ----
            for j in range(3):
                # fc1: h[ca, tok] = fc1_wT.T @ xT, into P1 padded interior
                for n in range(NCH):
                    ps = ps1p.tile([128, CHT], F32)
                    for k in range(6):
                        nc.tensor.matmul(
                            ps[:],
                            fc1w_sb[k][:, j * 128:(j + 1) * 128],
                            xT_sb[k][:, n * CHT:(n + 1) * CHT],
                            start=(k == 0), stop=(k == 5),
                        )
                    for q in range(2):
                        tpl = 2 * n + q
                        r0 = 16 * (tpl + 1) + 1
                        dst = _win(P1[j][:], 0).rearrange(
                            "p (r w) -> p r w", r=NROW, w=WP)[
                            :, r0:r0 + 14, 2:16]
                        src = ps[:, q * HW:(q + 1) * HW].rearrange(
                            "p (h w) -> p h w", h=H, w=W)
                        nc.scalar.copy(dst, src)

                # stage 1 PE partial: diag(w) matmuls for S_PE taps; ACT
                # merges (+bias1) into P2
                if use_pe:
                    for c in range(8):
                        pc = pscp.tile([128, 512], F32)
                        for i, ti in enumerate(S_PE):
                            dt, dh, dw = TAPS1[ti]
                            off = dt * 256 + dh * 16 + dw
                            nc.tensor.matmul(
                                pc[:], dg1[(j, ti)][:],
                                regwin(P1[j][:], off, c),
                                start=(i == 0), stop=(i == len(S_PE) - 1),
                            )
                        nc.scalar.activation(
                            regwin(P2[j][:], 0, c), pc[:],
                            IDENT, bias=b1_sb[j][:], scale=1.0)

                # stage 1 DVE taps: two independent half-region chains
                for h in range(2):
                    first_dve = not use_pe
                    acc = reghalf(P2[j][:], 0, h)
                    for idx in s_dve:
                        dt, dh, dw = TAPS1[idx]
                        off = dt * 256 + dh * 16 + dw
                        src = reghalf(P1[j][:], off, h)
                        if first_dve:
                            nc.vector.scalar_tensor_tensor(
                                acc, src, t1_sb[j][:, idx:idx + 1],
                                b1_sb[j][:, 0:1].broadcast_to([128, 128, 14]),
                                op0=mult, op1=add)
                            first_dve = False
                        else:
                            nc.vector.scalar_tensor_tensor(
                                acc, src, t1_sb[j][:, idx:idx + 1], acc,
                                op0=mult, op1=add)

                # stage 2 on PE: Ft = conv3_T(P2) + bias2, merged per-plane
                # from PSUM straight into the tight token layout
                for c in range(8):
                    pc = pscp.tile([128, 512], F32)
                    for i, dt in enumerate((-1, 0, 1)):
                        nc.tensor.matmul(
                            pc[:], dg2[(j, dt + 1)][:],
                            regwin(P2[j][:], dt * 256, c),
                            start=(i == 0), stop=(i == 2),
                        )
                    for q in range(2):
                        tpl = 2 * c + q
                        dst = Ft[j][:, tpl * HW:(tpl + 1) * HW].rearrange(
                            "p (h w) -> p h w", h=H, w=W)
                        src = pc[:, q * 256:(q + 1) * 256].rearrange(
                            "p (h w) -> p h w", h=16, w=16)[:, 1:15, 2:16]
                        nc.scalar.activation(
                            dst, src, IDENT, bias=b2_sb[j][:], scale=1.0)

            # ---- fc2 + residual add (identity matmul) + store ----
            m_tiles = [(m * 128, 128) for m in range(24)] + [(3072, 64)]
            for (m0, M) in m_tiles:
                xm = xmp.tile([128, C], BF)
                nc.sync.dma_start(out=xm[:M], in_=xtok[m0:m0 + M, :])
                ot = outp.tile([128, C], F32)
                for nh in range(2):
                    ps = ps2p.tile([128, 384], F32)
                    for k in range(3):
                        nc.tensor.matmul(
                            ps[:M],
                            Ft[k][:, m0:m0 + M],
                            fc2w_sb[k][:, nh * 384:(nh + 1) * 384],
                            start=(k == 0), stop=False,
                        )
                    nc.tensor.matmul(
                        ps[:M], ident[:M, :M],
                        xm[:M, nh * 384:(nh + 1) * 384],
                        start=False, stop=True,
                    )
                    nc.scalar.copy(ot[:M, nh * 384:(nh + 1) * 384], ps[:M])
                nc.sync.dma_start(out=out[m0:m0 + M, :], in_=ot[:M])

    nc.compile()
    return nc


def _prep_shared(fc1_w, fc1_b, conv1_w, conv1_b, conv2_w, conv2_b,
                 conv3_w, conv3_b, proj_w, proj_b, fc2_w, fc2_b):
    assert not np.any(fc1_b), "nonzero fc1_b not supported by this build"
    # merged stage-1 kernel: (c1 + c2 + c3)/3 + identity
    w_eff = np.array(conv3_w[:, 0], dtype=np.float64)            # [CA,3,3,3]
    w_eff[:, :, 1, 1] += conv1_w[:, 0, :, 0, 0]
    w_eff[:, 1, :, :] += conv2_w[:, 0, 0, :, :]
    w_eff /= 3.0
    w_eff[:, 1, 1, 1] += 1.0
    b_eff = (conv1_b + conv2_b + conv3_b) / 3.0
    # stage-2 (proj) taps along T + identity
    a_eff = np.array(proj_w[:, 0, :, 0, 0], dtype=np.float64)    # [CA,3]
    a_eff[:, 1] += 1.0

    taps1 = np.empty((3, 128, 27), np.float32)
    for idx, (dt, dh, dw) in enumerate(TAPS1):
        taps1[:, :, idx] = w_eff[:, dt + 1, dh + 1, dw + 1].reshape(3, 128)
    bias1 = np.asarray(b_eff, np.float32).reshape(3, 128, 1)
    taps2 = np.asarray(a_eff, np.float32).reshape(3, 128, 3)
    bias2 = np.asarray(proj_b, np.float32).reshape(3, 128, 1)

    idx128 = np.arange(128)
    sel = S_PE if S_PE else [0]
    diag1 = np.zeros((3, len(sel), 128, 128), np.float32)
    diag1[:, :, idx128, idx128] = taps1[:, :, sel].transpose(0, 2, 1)
    diag2 = np.zeros((3, 3, 128, 128), np.float32)
    diag2[:, :, idx128, idx128] = taps2.transpose(0, 2, 1)

    fc1_wT = np.ascontiguousarray(np.asarray(fc1_w, np.float32).T).astype(BF16)
    fc2_wT = np.ascontiguousarray(np.asarray(fc2_w, np.float32).T).astype(BF16)
    return dict(fc1_wT=fc1_wT, fc2_wT=fc2_wT, taps1=taps1, bias1=bias1,
                bias2=bias2, diag1=diag1.astype(BF16),
                diag2=diag2.astype(BF16)), np.asarray(fc2_b, np.float32)


def kernel(x, fc1_w, fc1_b, conv1_w, conv1_b, conv2_w, conv2_b,
           conv3_w, conv3_b, proj_w, proj_b, fc2_w, fc2_b, T=16):
    global LAST_EXEC_NS, LAST_RESULTS
    x = np.asarray(x, np.float32)
    Tv = int(np.asarray(T))
    assert Tv == 16 and x.shape == (B * Tv, L, C), (Tv, x.shape)

    if "nc" not in _CACHE:
        _CACHE["nc"] = _build_graph()
    nc = _CACHE["nc"]

    shared, fc2_b_np = _prep_shared(
        np.asarray(fc1_w, np.float32), np.asarray(fc1_b, np.float32),
        np.asarray(conv1_w, np.float32), np.asarray(conv1_b, np.float32),
        np.asarray(conv2_w, np.float32), np.asarray(conv2_b, np.float32),
        np.asarray(conv3_w, np.float32), np.asarray(conv3_b, np.float32),
        np.asarray(proj_w, np.float32), np.asarray(proj_b, np.float32),
        np.asarray(fc2_w, np.float32), np.asarray(fc2_b, np.float32))

    in_maps = []
    for i in range(NCORES):
        clip = x[i * Tv:(i + 1) * Tv]                    # [16, 197, 768]
        tokens = np.ascontiguousarray(clip[:, 1:, :]).reshape(TOK, C)
        m = dict(shared)
        m["xT"] = np.ascontiguousarray(tokens.T).astype(BF16)
        m["xtok"] = (tokens + fc2_b_np[None, :]).astype(BF16)
        in_maps.append(m)

    res = run_bass_kernel_spmd(nc, in_maps, core_ids=list(range(NCORES)),
                               trace=TRACE)
    LAST_EXEC_NS = res.exec_time_ns
    LAST_RESULTS = res

    full = np.array(x)  # CLS rows (and everything) start as x
    for i in range(NCORES):
        h = res.results[i]["out"].reshape(Tv, HW, C)
        full[i * Tv:(i + 1) * Tv, 1:, :] = h
    return full


# revision 12
# speedup vs baseline: 2.0681x; 1.1382x over previous
"""Trainium2 Bass kernel for nn_Adapter (ViT video adapter block).

Reference computation (per clip of T=16 frames, 14x14 spatial, 768 ch):
  h   = fc1(x_tokens)                                  # [3136, 384]
  g   = (dw3d_311(h) + dw3d_133(h) + dw3d_333(h))/3 + h
  f   = g + dw3d_311(g)            (proj)
  out = x_tokens + fc2(f)
CLS token passes through unchanged.

Strategy: data-parallel over the 8 clips (B=8), one clip per NeuronCore.
On-chip layout is channel-major [Ca, T*H*W] so the depthwise convs become
per-partition shift-and-MAC chains; the three 3D convs of stage 1 merge
into one 27-tap kernel (+identity), proj is 3 taps along T (+identity).
Spatial data sits in a zero-halo padded [18,16,16] flat layout; a shifted
tap is just a flat window offset (edge reads land in zero halos).  Taps are
split between VectorE (fused scalar_tensor_tensor MACs, bf16 2x mode — odd
offsets read a one-element-shifted copy P1b to stay 4B-aligned) and
TensorE (diag(w) matmuls accumulating in PSUM).  fc1/fc2 run on TensorE in
bf16; the residual x-add rides the fc2 PSUM group as an identity matmul.
"""

import os
import sys

sys.path.insert(0, "/opt/trn_rl_repo")

import numpy as np
import ml_dtypes

import concourse.bass as bass
import concourse.mybir as mybir
from concourse import bacc
from concourse.tile import TileContext
from concourse.bass_utils import run_bass_kernel_spmd
from concourse.masks import make_identity

BF16 = ml_dtypes.bfloat16

# Problem geometry (hardcoded; kernel must be self-contained).
B, T, L, C, CA = 8, 16, 197, 768, 384
H = W = 14
HW = H * W            # 196
TOK = T * HW          # 3136 tokens per clip
NCORES = 8
# padded conv layout: [Tp, Hp, Wp] flat with zero halos
TP, HP, WP = 18, 16, 16
NROW = TP * HP        # 288 rows of 16
PADSZ = TP * HP * WP  # 4608
PAD0 = 288            # lead/trail zero pad so shifted windows stay in-bounds
PADE = PAD0 + PADSZ + PAD0  # 5184 allocated
# fc1 output chunking: 392 tokens = 2 t-planes per PSUM bank
NCH, CHT = 8, 392

F32 = mybir.dt.float32
BF = mybir.dt.bfloat16

# stage-1 tap enumeration (27 taps)
TAPS1 = [(dt, dh, dw) for dt in (-1, 0, 1) for dh in (-1, 0, 1) for dw in (-1, 0, 1)]
TAP0 = TAPS1.index((0, 0, 0))

# ---- engine split config ----
# stage-1 tap indices computed on TensorE as diag(w) matmuls (must include
# TAP0 when non-empty so the bias rides the PSUM->SBUF merge).
N_PE_TAPS = int(os.environ.get("KN_PE_TAPS", "15"))
S_PE = ([TAP0] + [i for i in range(27) if i != TAP0])[:N_PE_TAPS]

_CACHE = {}

TRACE = os.environ.get("BASS_KERNEL_TRACE", "0") == "1"
LAST_EXEC_NS = None
LAST_RESULTS = None


def _win(ap, off, lo=0, hi=PADSZ):
    """Flat shifted window [128, hi-lo] of a [128, PADE] padded tile."""
    return ap[:, PAD0 + off + lo:PAD0 + off + hi]


def _rows(ap, off):
    """Shifted window as [128, NROW, 14] (skips the 2 halo cols per row)."""
    w = _win(ap, off)
    return w.rearrange("p (r w) -> p r w", r=NROW, w=WP)[:, :, 2:16]


def _build_graph():
    nc = bacc.Bacc("TRN2", target_bir_lowering=False, debug=False,
                   num_devices=NCORES)

    xT = nc.dram_tensor("xT", [C, TOK], BF, kind="ExternalInput").ap()
    xtok = nc.dram_tensor("xtok", [TOK, C], BF, kind="ExternalInput").ap()
    fc1_wT = nc.dram_tensor("fc1_wT", [C, CA], BF, kind="ExternalInput").ap()
    fc2_wT = nc.dram_tensor("fc2_wT", [CA, C], BF, kind="ExternalInput").ap()
    taps1 = nc.dram_tensor("taps1", [3, 128, 27], F32, kind="ExternalInput").ap()
    bias1 = nc.dram_tensor("bias1", [3, 128, 1], F32, kind="ExternalInput").ap()
    bias2 = nc.dram_tensor("bias2", [3, 128, 1], F32, kind="ExternalInput").ap()
    diag1 = nc.dram_tensor("diag1", [3, max(1, len(S_PE)), 128, 128], BF,
                           kind="ExternalInput").ap()
    diag2 = nc.dram_tensor("diag2", [3, 3, 128, 128], BF,
                           kind="ExternalInput").ap()
    out = nc.dram_tensor("out", [TOK, C], F32, kind="ExternalOutput").ap()

    mult = mybir.AluOpType.mult
    add = mybir.AluOpType.add
    IDENT = mybir.ActivationFunctionType.Identity

    use_pe = len(S_PE) > 0
    s_dve = [i for i in range(27) if i not in S_PE]
    # conv "region": planes 1..16 of the padded layout = [256, 4352) flat,
    # 8 chunks of 512 (all interior tokens + in-plane halo rows/cols).
    REG0 = 256

    def regwin(ap, off, c):
        lo = PAD0 + off + REG0 + 512 * c
        return ap[:, lo:lo + 512]

    def reghalf(ap, off, h):
        """Half-region rows view [128, 128, 14] (rows 128h..128h+128)."""
        lo = PAD0 + off + REG0 + 2048 * h
        w = ap[:, lo:lo + 2048]
        return w.rearrange("p (r w) -> p r w", r=128, w=WP)[:, :, 2:16]

    with TileContext(nc) as tc:
        with (
            tc.tile_pool(name="persist", bufs=1) as pp,
            tc.tile_pool(name="xm", bufs=3) as xmp,
            tc.tile_pool(name="outs", bufs=3) as outp,
            tc.tile_pool(name="ps1", bufs=2, space="PSUM") as ps1p,
            tc.tile_pool(name="ps2", bufs=3, space="PSUM") as ps2p,
            tc.tile_pool(name="psc", bufs=3, space="PSUM") as pscp,
        ):
            # ---- xT (channel-major tokens) + fc1 weights first: they gate
            # the first matmul ----
            xT_sb = []
            for k in range(6):
                t = pp.tile([128, TOK], BF, tag=f"xT{k}", name=f"xT{k}")
                nc.sync.dma_start(out=t[:, :TOK // 2],
                                  in_=xT[k * 128:(k + 1) * 128, :TOK // 2])
                nc.sync.dma_start(out=t[:, TOK // 2:],
                                  in_=xT[k * 128:(k + 1) * 128, TOK // 2:])
                xT_sb.append(t)
            fc1w_sb = []
            for k in range(6):
                t = pp.tile([128, CA], BF, tag=f"fc1w{k}", name=f"fc1w{k}")
                nc.sync.dma_start(out=t[:], in_=fc1_wT[k * 128:(k + 1) * 128, :])
                fc1w_sb.append(t)
            t1_sb, b1_sb, b2_sb = [], [], []
            for j in range(3):
                a = pp.tile([128, 27], F32, tag=f"t1_{j}", name=f"t1_{j}")
                nc.sync.dma_start(out=a[:], in_=taps1[j])
                t1_sb.append(a)
                a = pp.tile([128, 1], F32, tag=f"b1_{j}", name=f"b1_{j}")
                nc.sync.dma_start(out=a[:], in_=bias1[j])
                b1_sb.append(a)
                a = pp.tile([128, 1], F32, tag=f"b2_{j}", name=f"b2_{j}")
                nc.sync.dma_start(out=a[:], in_=bias2[j])
                b2_sb.append(a)

            # diag tap matrices (host-built), identity for the x-add
            dg1, dg2 = {}, {}
            for j in range(3):
                for i, ti in enumerate(S_PE):
                    d = pp.tile([128, 128], BF, tag=f"dg{j}_{ti}",
                                name=f"dg{j}_{ti}")
                    nc.sync.dma_start(out=d[:], in_=diag1[j, i])
                    dg1[(j, ti)] = d
                for ti in range(3):
                    d = pp.tile([128, 128], BF, tag=f"dh{j}_{ti}",
                                name=f"dh{j}_{ti}")
                    nc.sync.dma_start(out=d[:], in_=diag2[j, ti])
                    dg2[(j, ti)] = d
            ident = pp.tile([128, 128], BF, tag="ident", name="ident")
            make_identity(nc, ident[:])
            fc2w_sb = []
            for k in range(3):
                t = pp.tile([128, C], BF, tag=f"fc2w{k}", name=f"fc2w{k}")
                nc.sync.dma_start(out=t[:], in_=fc2_wT[k * 128:(k + 1) * 128, :])
                fc2w_sb.append(t)

            # ---- conv buffers; only halos need zeroing (ScalarE memzero,
            # DVE stays free) ----
            P1 = [pp.tile([128, PADE], BF, tag=f"P1_{j}", name=f"P1_{j}")
                  for j in range(3)]
            P2 = [pp.tile([128, PADE], BF, tag=f"P2_{j}", name=f"P2_{j}")
                  for j in range(3)]
            Ft = [pp.tile([128, TOK], BF, tag=f"Ft_{j}", name=f"Ft_{j}")
                  for j in range(3)]
            for j in range(3):
                nc.scalar.memzero(P1[j][:])
                # stage-2 windows only read P2 planes 0 / 17 outside the
                # written region
                nc.scalar.memzero(P2[j][:, PAD0:PAD0 + 256])
                nc.scalar.memzero(P2[j][:, PAD0 + 4352:PAD0 + 4608])

            # ---- per-ca-tile pipeline: fc1 -> conv stage 1 (PE+DVE) ----
            # stage2(j) is emitted after conv(j+1) so the PE keeps feeding
            # the DVE chains without interruption.
            def fc1_block(j):
                for n in range(NCH):
                    ps = ps1p.tile([128, CHT], F32, tag="ps1t", name=f"ps1_{j}_{n}")
                    for k in range(6):
                        nc.tensor.matmul(
                            ps[:],
                            fc1w_sb[k][:, j * 128:(j + 1) * 128],
                            xT_sb[k][:, n * CHT:(n + 1) * CHT],
                            start=(k == 0), stop=(k == 5),
                        )
                    for q in range(2):
                        tpl = 2 * n + q
                        r0 = 16 * (tpl + 1) + 1
                        dst = _win(P1[j][:], 0).rearrange(
                            "p (r w) -> p r w", r=NROW, w=WP)[
                            :, r0:r0 + 14, 2:16]
                        src = ps[:, q * HW:(q + 1) * HW].rearrange(
                            "p (h w) -> p h w", h=H, w=W)
                        nc.scalar.copy(dst, src)

            def conv_block(j):
                if use_pe:
                    for c in range(8):
                        pc = pscp.tile([128, 512], F32, tag="psct", name=f"pc1_{j}_{c}")
                        for i, ti in enumerate(S_PE):
                            dt, dh, dw = TAPS1[ti]
                            off = dt * 256 + dh * 16 + dw
                            nc.tensor.matmul(
                                pc[:], dg1[(j, ti)][:],
                                regwin(P1[j][:], off, c),
                                start=(i == 0), stop=(i == len(S_PE) - 1),
                            )
                        nc.scalar.activation(
                            regwin(P2[j][:], 0, c), pc[:],
                            IDENT, bias=b1_sb[j][:], scale=1.0)
                for h in range(2):
                    first_dve = not use_pe
                    acc = reghalf(P2[j][:], 0, h)
                    for idx in s_dve:
                        dt, dh, dw = TAPS1[idx]
                        off = dt * 256 + dh * 16 + dw
                        src = reghalf(P1[j][:], off, h)
                        if first_dve:
                            nc.vector.scalar_tensor_tensor(
                                acc, src, t1_sb[j][:, idx:idx + 1],
                                b1_sb[j][:, 0:1].broadcast_to([128, 128, 14]),
                                op0=mult, op1=add)
                            first_dve = False
                        else:
                            nc.vector.scalar_tensor_tensor(
                                acc, src, t1_sb[j][:, idx:idx + 1], acc,
                                op0=mult, op1=add)

            def stage2_block(j):
                for c in range(8):
                    pc = pscp.tile([128, 512], F32, tag="psct", name=f"pc2_{j}_{c}")
                    for i, dt in enumerate((-1, 0, 1)):
                        nc.tensor.matmul(
                            pc[:], dg2[(j, dt + 1)][:],
                            regwin(P2[j][:], dt * 256, c),
                            start=(i == 0), stop=(i == 2),
                        )
                    for q in range(2):
                        tpl = 2 * c + q
                        dst = Ft[j][:, tpl * HW:(tpl + 1) * HW].rearrange(
                            "p (h w) -> p h w", h=H, w=W)
                        src = pc[:, q * 256:(q + 1) * 256].rearrange(
                            "p (h w) -> p h w", h=16, w=16)[:, 1:15, 2:16]
                        nc.scalar.activation(
                            dst, src, IDENT, bias=b2_sb[j][:], scale=1.0)

            fc1_block(0)
            conv_block(0)
            fc1_block(1)
            conv_block(1)
            stage2_block(0)
            fc1_block(2)
            conv_block(2)
            stage2_block(1)
            stage2_block(2)

            # ---- fc2 + residual add (identity matmul) + store ----
            m_tiles = [(m * 128, 128) for m in range(24)] + [(3072, 64)]
            for (m0, M) in m_tiles:
                xm = xmp.tile([128, C], BF)
                nc.sync.dma_start(out=xm[:M], in_=xtok[m0:m0 + M, :])
                ot = outp.tile([128, C], F32)
                for nh in range(2):
                    ps = ps2p.tile([128, 384], F32)
                    for k in range(3):
                        nc.tensor.matmul(
                            ps[:M],
                            Ft[k][:, m0:m0 + M],
                            fc2w_sb[k][:, nh * 384:(nh + 1) * 384],
                            start=(k == 0), stop=False,
                        )
                    nc.tensor.matmul(
                        ps[:M], ident[:M, :M],
                        xm[:M, nh * 384:(nh + 1) * 384],
                        start=False, stop=True,
                    )
                    nc.scalar.copy(ot[:M, nh * 384:(nh + 1) * 384], ps[:M])
                nc.sync.dma_start(out=out[m0:m0 + M, :], in_=ot[:M])

    nc.compile()
    return nc


def _prep_shared(fc1_w, fc1_b, conv1_w, conv1_b, conv2_w, conv2_b,
                 conv3_w, conv3_b, proj_w, proj_b, fc2_w, fc2_b):
    assert not np.any(fc1_b), "nonzero fc1_b not supported by this build"
    # merged stage-1 kernel: (c1 + c2 + c3)/3 + identity
    w_eff = np.array(conv3_w[:, 0], dtype=np.float64)            # [CA,3,3,3]
    w_eff[:, :, 1, 1] += conv1_w[:, 0, :, 0, 0]
    w_eff[:, 1, :, :] += conv2_w[:, 0, 0, :, :]
    w_eff /= 3.0
    w_eff[:, 1, 1, 1] += 1.0
    b_eff = (conv1_b + conv2_b + conv3_b) / 3.0
    # stage-2 (proj) taps along T + identity
    a_eff = np.array(proj_w[:, 0, :, 0, 0], dtype=np.float64)    # [CA,3]
    a_eff[:, 1] += 1.0

    taps1 = np.empty((3, 128, 27), np.float32)
    for idx, (dt, dh, dw) in enumerate(TAPS1):
        taps1[:, :, idx] = w_eff[:, dt + 1, dh + 1, dw + 1].reshape(3, 128)
    bias1 = np.asarray(b_eff, np.float32).reshape(3, 128, 1)
    taps2 = np.asarray(a_eff, np.float32).reshape(3, 128, 3)
    bias2 = np.asarray(proj_b, np.float32).reshape(3, 128, 1)

    idx128 = np.arange(128)
    sel = S_PE if S_PE else [0]
    diag1 = np.zeros((3, len(sel), 128, 128), np.float32)
    diag1[:, :, idx128, idx128] = taps1[:, :, sel].transpose(0, 2, 1)
    diag2 = np.zeros((3, 3, 128, 128), np.float32)
    diag2[:, :, idx128, idx128] = taps2.transpose(0, 2, 1)

    fc1_wT = np.ascontiguousarray(np.asarray(fc1_w, np.float32).T).astype(BF16)
    fc2_wT = np.ascontiguousarray(np.asarray(fc2_w, np.float32).T).astype(BF16)
    return dict(fc1_wT=fc1_wT, fc2_wT=fc2_wT, taps1=taps1, bias1=bias1,
                bias2=bias2, diag1=diag1.astype(BF16),
                diag2=diag2.astype(BF16)), np.asarray(fc2_b, np.float32)


def kernel(x, fc1_w, fc1_b, conv1_w, conv1_b, conv2_w, conv2_b,
           conv3_w, conv3_b, proj_w, proj_b, fc2_w, fc2_b, T=16):
    global LAST_EXEC_NS, LAST_RESULTS
    x = np.asarray(x, np.float32)
    Tv = int(np.asarray(T))
    assert Tv == 16 and x.shape == (B * Tv, L, C), (Tv, x.shape)

    if "nc" not in _CACHE:
        _CACHE["nc"] = _build_graph()
    nc = _CACHE["nc"]

    shared, fc2_b_np = _prep_shared(
        np.asarray(fc1_w, np.float32), np.asarray(fc1_b, np.float32),
        np.asarray(conv1_w, np.float32), np.asarray(conv1_b, np.float32),
        np.asarray(conv2_w, np.float32), np.asarray(conv2_b, np.float32),
        np.asarray(conv3_w, np.float32), np.asarray(conv3_b, np.float32),
        np.asarray(proj_w, np.float32), np.asarray(proj_b, np.float32),
        np.asarray(fc2_w, np.float32), np.asarray(fc2_b, np.float32))

    in_maps = []
    for i in range(NCORES):
        clip = x[i * Tv:(i + 1) * Tv]                    # [16, 197, 768]
        tokens = np.ascontiguousarray(clip[:, 1:, :]).reshape(TOK, C)
        m = dict(shared)
        m["xT"] = np.ascontiguousarray(tokens.T).astype(BF16)
        m["xtok"] = (tokens + fc2_b_np[None, :]).astype(BF16)
        in_maps.append(m)

    res = run_bass_kernel_spmd(nc, in_maps, core_ids=list(range(NCORES)),
                               trace=TRACE)
    LAST_EXEC_NS = res.exec_time_ns
    LAST_RESULTS = res

    full = np.array(x)  # CLS rows (and everything) start as x
    for i in range(NCORES):
        h = res.results[i]["out"].reshape(Tv, HW, C)
        full[i * Tv:(i + 1) * Tv, 1:, :] = h
    return full


# revision 16
# speedup vs baseline: 2.1379x; 1.0337x over previous
"""Trainium2 Bass kernel for nn_Adapter (ViT video adapter block).

Reference computation (per clip of T=16 frames, 14x14 spatial, 768 ch):
  h   = fc1(x_tokens)                                  # [3136, 384]
  g   = (dw3d_311(h) + dw3d_133(h) + dw3d_333(h))/3 + h
  f   = g + dw3d_311(g)            (proj)
  out = x_tokens + fc2(f)
CLS token passes through unchanged.

Strategy: data-parallel over the 8 clips (B=8), one clip per NeuronCore.
On-chip layout is channel-major [Ca, T*H*W] so the depthwise convs become
per-partition shift-and-MAC chains; the three 3D convs of stage 1 merge
into one 27-tap kernel (+identity), proj is 3 taps along T (+identity).
Spatial data sits in a zero-halo padded [18,16,16] flat layout; a shifted
tap is just a flat window offset (edge reads land in zero halos).  Taps are
split between VectorE (fused scalar_tensor_tensor MACs, bf16 2x mode — odd
offsets read a one-element-shifted copy P1b to stay 4B-aligned) and
TensorE (diag(w) matmuls accumulating in PSUM).  fc1/fc2 run on TensorE in
bf16; the residual x-add rides the fc2 PSUM group as an identity matmul.
"""

import os
import sys

sys.path.insert(0, "/opt/trn_rl_repo")

import numpy as np
import ml_dtypes

import concourse.bass as bass
import concourse.mybir as mybir
from concourse import bacc
from concourse.tile import TileContext
from concourse.bass_utils import run_bass_kernel_spmd
from concourse.masks import make_identity

BF16 = ml_dtypes.bfloat16

# Problem geometry (hardcoded; kernel must be self-contained).
B, T, L, C, CA = 8, 16, 197, 768, 384
H = W = 14
HW = H * W            # 196
TOK = T * HW          # 3136 tokens per clip
NCORES = 8
# padded conv layout: [Tp, Hp, Wp] flat with shared zero halos:
# each 15-wide row = [halo, 14 data]; each 15-row plane = [halo row, 14 data
# rows]; t has dedicated halo planes 0 and 17.  Any shifted read that runs
# off an edge provably lands on a halo cell or the lead/trail pad.
TP, HP, WP = 18, 15, 15
PLANE = HP * WP       # 225
NROW = TP * HP        # 270 rows of 15
PADSZ = TP * PLANE    # 4050
CHUNK = 2 * PLANE     # 450 = one PSUM-chunk (2 planes)
HALFR = 4 * CHUNK     # 1800 flat = half the 16-plane region
PAD0 = 288            # lead/trail zero pad so shifted windows stay in-bounds
PADE = PAD0 + PADSZ + PAD0
# fc1 output chunking: 392 tokens = 2 t-planes per PSUM bank
NCH, CHT = 8, 392

F32 = mybir.dt.float32
BF = mybir.dt.bfloat16

# stage-1 tap enumeration (27 taps)
TAPS1 = [(dt, dh, dw) for dt in (-1, 0, 1) for dh in (-1, 0, 1) for dw in (-1, 0, 1)]
TAP0 = TAPS1.index((0, 0, 0))

# ---- engine split config ----
# stage-1 tap indices computed on TensorE as diag(w) matmuls (must include
# TAP0 when non-empty so the bias rides the PSUM->SBUF merge).
N_PE_TAPS = int(os.environ.get("KN_PE_TAPS", "15"))
S_PE = ([TAP0] + [i for i in range(27) if i != TAP0])[:N_PE_TAPS]

_CACHE = {}

TRACE = os.environ.get("BASS_KERNEL_TRACE", "0") == "1"
LAST_EXEC_NS = None
LAST_RESULTS = None


def _win(ap, off, lo=0, hi=PADSZ):
    """Flat shifted window [128, hi-lo] of a [128, PADE] padded tile."""
    return ap[:, PAD0 + off + lo:PAD0 + off + hi]


def _build_graph():
    nc = bacc.Bacc("TRN2", target_bir_lowering=False, debug=False,
                   num_devices=NCORES)

    xT = nc.dram_tensor("xT", [C, TOK], BF, kind="ExternalInput").ap()
    xtok = nc.dram_tensor("xtok", [TOK, C], BF, kind="ExternalInput").ap()
    fc1_wT = nc.dram_tensor("fc1_wT", [C, CA], BF, kind="ExternalInput").ap()
    fc2_wT = nc.dram_tensor("fc2_wT", [CA, C], BF, kind="ExternalInput").ap()
    taps1 = nc.dram_tensor("taps1", [3, 128, 27], F32, kind="ExternalInput").ap()
    bias1 = nc.dram_tensor("bias1", [3, 128, 1], F32, kind="ExternalInput").ap()
    bias2 = nc.dram_tensor("bias2", [3, 128, 1], F32, kind="ExternalInput").ap()
    diag1 = nc.dram_tensor("diag1", [3, max(1, len(S_PE)), 128, 128], BF,
                           kind="ExternalInput").ap()
    diag2 = nc.dram_tensor("diag2", [3, 3, 128, 128], BF,
                           kind="ExternalInput").ap()
    out = nc.dram_tensor("out", [TOK, C], F32, kind="ExternalOutput").ap()

    mult = mybir.AluOpType.mult
    add = mybir.AluOpType.add
    IDENT = mybir.ActivationFunctionType.Identity

    use_pe = len(S_PE) > 0
    s_dve = [i for i in range(27) if i not in S_PE]
    # conv "region": planes 1..16 of the padded layout, 8 chunks of CHUNK
    # (all interior tokens + in-plane halo rows/cols).
    REG0 = PLANE

    def regwin(ap, off, c):
        lo = PAD0 + off + REG0 + CHUNK * c
        return ap[:, lo:lo + CHUNK]

    def reghalf(ap, off, h):
        """Half-region rows view [128, 120, 14] (skipping halo col)."""
        lo = PAD0 + off + REG0 + HALFR * h
        w = ap[:, lo:lo + HALFR]
        return w.rearrange("p (r w) -> p r w", r=HALFR // WP, w=WP)[:, :, 1:15]

    with TileContext(nc) as tc:
        with (
            tc.tile_pool(name="persist", bufs=1) as pp,
            tc.tile_pool(name="xm", bufs=3) as xmp,
            tc.tile_pool(name="outs", bufs=3) as outp,
            tc.tile_pool(name="ps1", bufs=2, space="PSUM") as ps1p,
            tc.tile_pool(name="ps2", bufs=3, space="PSUM") as ps2p,
            tc.tile_pool(name="psc", bufs=3, space="PSUM") as pscp,
        ):
            # ---- xT (channel-major tokens) + fc1 weights first: they gate
            # the first matmul ----
            xT_sb = []
            for k in range(6):
                t = pp.tile([128, TOK], BF, tag=f"xT{k}", name=f"xT{k}")
                nc.sync.dma_start(out=t[:, :TOK // 2],
                                  in_=xT[k * 128:(k + 1) * 128, :TOK // 2])
                nc.sync.dma_start(out=t[:, TOK // 2:],
                                  in_=xT[k * 128:(k + 1) * 128, TOK // 2:])
                xT_sb.append(t)
            fc1w_sb = []
            for k in range(6):
                t = pp.tile([128, CA], BF, tag=f"fc1w{k}", name=f"fc1w{k}")
                nc.sync.dma_start(out=t[:], in_=fc1_wT[k * 128:(k + 1) * 128, :])
                fc1w_sb.append(t)
            t1_sb, b1_sb, b2_sb = [], [], []
            for j in range(3):
                a = pp.tile([128, 27], F32, tag=f"t1_{j}", name=f"t1_{j}")
                nc.sync.dma_start(out=a[:], in_=taps1[j])
                t1_sb.append(a)
                a = pp.tile([128, 1], F32, tag=f"b1_{j}", name=f"b1_{j}")
                nc.sync.dma_start(out=a[:], in_=bias1[j])
                b1_sb.append(a)
                a = pp.tile([128, 1], F32, tag=f"b2_{j}", name=f"b2_{j}")
                nc.sync.dma_start(out=a[:], in_=bias2[j])
                b2_sb.append(a)

            # diag tap matrices (host-built), identity for the x-add
            dg1, dg2 = {}, {}
            for j in range(3):
                for i, ti in enumerate(S_PE):
                    d = pp.tile([128, 128], BF, tag=f"dg{j}_{ti}",
                                name=f"dg{j}_{ti}")
                    nc.sync.dma_start(out=d[:], in_=diag1[j, i])
                    dg1[(j, ti)] = d
                for ti in range(3):
                    d = pp.tile([128, 128], BF, tag=f"dh{j}_{ti}",
                                name=f"dh{j}_{ti}")
                    nc.sync.dma_start(out=d[:], in_=diag2[j, ti])
                    dg2[(j, ti)] = d
            ident = pp.tile([128, 128], BF, tag="ident", name="ident")
            make_identity(nc, ident[:])
            fc2w_sb = []
            for k in range(3):
                t = pp.tile([128, C], BF, tag=f"fc2w{k}", name=f"fc2w{k}")
                nc.sync.dma_start(out=t[:], in_=fc2_wT[k * 128:(k + 1) * 128, :])
                fc2w_sb.append(t)

            # ---- conv buffers; only halos need zeroing (ScalarE memzero,
            # DVE stays free) ----
            P1 = [pp.tile([128, PADE], BF, tag=f"P1_{j}", name=f"P1_{j}")
                  for j in range(3)]
            P2 = [pp.tile([128, PADE], BF, tag=f"P2_{j}", name=f"P2_{j}")
                  for j in range(3)]
            Ft = [pp.tile([128, TOK], BF, tag=f"Ft_{j}", name=f"Ft_{j}")
                  for j in range(3)]
            for j in range(3):
                nc.scalar.memzero(P1[j][:])
                # stage-2 windows only read P2 planes 0 / 17 outside the
                # written region
                nc.scalar.memzero(P2[j][:, PAD0 - 2:PAD0 + PLANE + 1])
                nc.scalar.memzero(P2[j][:, PAD0 + 17 * PLANE - 1:PAD0 + PADSZ])

            # ---- per-ca-tile pipeline: fc1 -> conv stage 1 (PE+DVE) ----
            # stage2(j) is emitted after conv(j+1) so the PE keeps feeding
            # the DVE chains without interruption.
            def fc1_block(j):
                for n in range(NCH):
                    ps = ps1p.tile([128, CHT], F32, tag="ps1t", name=f"ps1_{j}_{n}")
                    for k in range(6):
                        nc.tensor.matmul(
                            ps[:],
                            fc1w_sb[k][:, j * 128:(j + 1) * 128],
                            xT_sb[k][:, n * CHT:(n + 1) * CHT],
                            start=(k == 0), stop=(k == 5),
                        )
                    for q in range(2):
                        tpl = 2 * n + q
                        r0 = HP * (tpl + 1) + 1
                        dst = _win(P1[j][:], 0).rearrange(
                            "p (r w) -> p r w", r=NROW, w=WP)[
                            :, r0:r0 + 14, 1:15]
                        src = ps[:, q * HW:(q + 1) * HW].rearrange(
                            "p (h w) -> p h w", h=H, w=W)
                        nc.scalar.copy(dst, src)

            def conv_block(j):
                if use_pe:
                    for c in range(8):
                        pc = pscp.tile([128, CHUNK], F32, tag="psct", name=f"pc1_{j}_{c}")
                        for i, ti in enumerate(S_PE):
                            dt, dh, dw = TAPS1[ti]
                            off = dt * PLANE + dh * WP + dw
                            nc.tensor.matmul(
                                pc[:], dg1[(j, ti)][:],
                                regwin(P1[j][:], off, c),
                                start=(i == 0), stop=(i == len(S_PE) - 1),
                            )
                        nc.scalar.activation(
                            regwin(P2[j][:], 0, c), pc[:],
                            IDENT, bias=b1_sb[j][:], scale=1.0)
                for h in range(2):
                    first_dve = not use_pe
                    acc = reghalf(P2[j][:], 0, h)
                    for idx in s_dve:
                        dt, dh, dw = TAPS1[idx]
                        off = dt * PLANE + dh * WP + dw
                        src = reghalf(P1[j][:], off, h)
                        if first_dve:
                            nc.vector.scalar_tensor_tensor(
                                acc, src, t1_sb[j][:, idx:idx + 1],
                                b1_sb[j][:, 0:1].broadcast_to([128, HALFR // WP, 14]),
                                op0=mult, op1=add)
                            first_dve = False
                        else:
                            nc.vector.scalar_tensor_tensor(
                                acc, src, t1_sb[j][:, idx:idx + 1], acc,
                                op0=mult, op1=add)

            def stage2_block(j):
                for c in range(8):
                    pc = pscp.tile([128, CHUNK], F32, tag="psct", name=f"pc2_{j}_{c}")
                    for i, dt in enumerate((-1, 0, 1)):
                        nc.tensor.matmul(
                            pc[:], dg2[(j, dt + 1)][:],
                            regwin(P2[j][:], dt * PLANE, c),
                            start=(i == 0), stop=(i == 2),
                        )
                    for q in range(2):
                        tpl = 2 * c + q
                        dst = Ft[j][:, tpl * HW:(tpl + 1) * HW].rearrange(
                            "p (h w) -> p h w", h=H, w=W)
                        src = pc[:, q * PLANE:(q + 1) * PLANE].rearrange(
                            "p (h w) -> p h w", h=HP, w=WP)[:, 1:15, 1:15]
                        nc.scalar.activation(
                            dst, src, IDENT, bias=b2_sb[j][:], scale=1.0)

            fc1_block(0)
            conv_block(0)
            fc1_block(1)
            conv_block(1)
            stage2_block(0)
            fc1_block(2)
            conv_block(2)
            stage2_block(1)
            stage2_block(2)

            # ---- fc2 + residual add (identity matmul) + store ----
            m_tiles = [(m * 128, 128) for m in range(24)] + [(3072, 64)]
            for (m0, M) in m_tiles:
                xm = xmp.tile([128, C], BF)
                nc.sync.dma_start(out=xm[:M], in_=xtok[m0:m0 + M, :])
                ot = outp.tile([128, C], F32)
                for nh in range(2):
                    ps = ps2p.tile([128, 384], F32)
                    for k in range(3):
                        nc.tensor.matmul(
                            ps[:M],
                            Ft[k][:, m0:m0 + M],
                            fc2w_sb[k][:, nh * 384:(nh + 1) * 384],
                            start=(k == 0), stop=False,
                        )
                    nc.tensor.matmul(
                        ps[:M], ident[:M, :M],
                        xm[:M, nh * 384:(nh + 1) * 384],
                        start=False, stop=True,
                    )
                    nc.scalar.copy(ot[:M, nh * 384:(nh + 1) * 384], ps[:M])
                nc.sync.dma_start(out=out[m0:m0 + M, :], in_=ot[:M])

    nc.compile()
    return nc


def _prep_shared(fc1_w, fc1_b, conv1_w, conv1_b, conv2_w, conv2_b,
                 conv3_w, conv3_b, proj_w, proj_b, fc2_w, fc2_b):
    assert not np.any(fc1_b), "nonzero fc1_b not supported by this build"
    # merged stage-1 kernel: (c1 + c2 + c3)/3 + identity
    w_eff = np.array(conv3_w[:, 0], dtype=np.float64)            # [CA,3,3,3]
    w_eff[:, :, 1, 1] += conv1_w[:, 0, :, 0, 0]
    w_eff[:, 1, :, :] += conv2_w[:, 0, 0, :, :]
    w_eff /= 3.0
    w_eff[:, 1, 1, 1] += 1.0
    b_eff = (conv1_b + conv2_b + conv3_b) / 3.0
    # stage-2 (proj) taps along T + identity
    a_eff = np.array(proj_w[:, 0, :, 0, 0], dtype=np.float64)    # [CA,3]
    a_eff[:, 1] += 1.0

    taps1 = np.empty((3, 128, 27), np.float32)
    for idx, (dt, dh, dw) in enumerate(TAPS1):
        taps1[:, :, idx] = w_eff[:, dt + 1, dh + 1, dw + 1].reshape(3, 128)
    bias1 = np.asarray(b_eff, np.float32).reshape(3, 128, 1)
    taps2 = np.asarray(a_eff, np.float32).reshape(3, 128, 3)
    bias2 = np.asarray(proj_b, np.float32).reshape(3, 128, 1)

    idx128 = np.arange(128)
    sel = S_PE if S_PE else [0]
    diag1 = np.zeros((3, len(sel), 128, 128), np.float32)
    diag1[:, :, idx128, idx128] = taps1[:, :, sel].transpose(0, 2, 1)
    diag2 = np.zeros((3, 3, 128, 128), np.float32)
    diag2[:, :, idx128, idx128] = taps2.transpose(0, 2, 1)

    fc1_wT = np.ascontiguousarray(np.asarray(fc1_w, np.float32).T).astype(BF16)
    fc2_wT = np.ascontiguousarray(np.asarray(fc2_w, np.float32).T).astype(BF16)
    return dict(fc1_wT=fc1_wT, fc2_wT=fc2_wT, taps1=taps1, bias1=bias1,
                bias2=bias2, diag1=diag1.astype(BF16),
                diag2=diag2.astype(BF16)), np.asarray(fc2_b, np.float32)


def kernel(x, fc1_w, fc1_b, conv1_w, conv1_b, conv2_w, conv2_b,
           conv3_w, conv3_b, proj_w, proj_b, fc2_w, fc2_b, T=16):
    global LAST_EXEC_NS, LAST_RESULTS
    x = np.asarray(x, np.float32)
    Tv = int(np.asarray(T))
    assert Tv == 16 and x.shape == (B * Tv, L, C), (Tv, x.shape)

    if "nc" not in _CACHE:
        _CACHE["nc"] = _build_graph()
    nc = _CACHE["nc"]

    shared, fc2_b_np = _prep_shared(
        np.asarray(fc1_w, np.float32), np.asarray(fc1_b, np.float32),
        np.asarray(conv1_w, np.float32), np.asarray(conv1_b, np.float32),
        np.asarray(conv2_w, np.float32), np.asarray(conv2_b, np.float32),
        np.asarray(conv3_w, np.float32), np.asarray(conv3_b, np.float32),
        np.asarray(proj_w, np.float32), np.asarray(proj_b, np.float32),
        np.asarray(fc2_w, np.float32), np.asarray(fc2_b, np.float32))

    in_maps = []
    for i in range(NCORES):
        clip = x[i * Tv:(i + 1) * Tv]                    # [16, 197, 768]
        tokens = np.ascontiguousarray(clip[:, 1:, :]).reshape(TOK, C)
        m = dict(shared)
        m["xT"] = np.ascontiguousarray(tokens.T).astype(BF16)
        m["xtok"] = (tokens + fc2_b_np[None, :]).astype(BF16)
        in_maps.append(m)

    res = run_bass_kernel_spmd(nc, in_maps, core_ids=list(range(NCORES)),
                               trace=TRACE)
    LAST_EXEC_NS = res.exec_time_ns
    LAST_RESULTS = res

    full = np.array(x)  # CLS rows (and everything) start as x
    for i in range(NCORES):
        h = res.results[i]["out"].reshape(Tv, HW, C)
        full[i * Tv:(i + 1) * Tv, 1:, :] = h
    return full
